# revision 6
# baseline (speedup 1.0000x reference)
"""CPC spatial BCE loss kernel for 8 TRN2 NeuronCores.

Computation: loss = BCE(sigmoid((V1.reshape(N,D) @ V2.reshape(N,D).T) / D), eye(N))
with N=256, D=64*64*64=262144.

Strategy (memory-regime): shard the contraction dim D across the 8 cores
(64 MB of fp32 input per core -- the minimal possible HBM traffic). Each
core computes a partial Gram matrix [256, 256] over its 32768-wide slice
of D via TensorE matmuls accumulated in fp32 PSUM. The host lays out each
core's chunk d-major and pre-tiled ([nchunk, 128, mb, 256], i.e. exactly
the SBUF tile layout) so every chunk DMA is one fully-contiguous read
with the contraction dim landing on SBUF partitions -- no on-device
transposes. Loads are SWDGE cast-DMAs (fp32 HBM -> bf16 SBUF) so the
matmuls run single-pass bf16 (fp32 matmul on trn2 costs 4x: two HI/LO
passes at half streaming rate); accumulation stays fp32 in PSUM, so the
loss error is ~1e-5, far inside tolerance.

The partial Gram matrices are summed on the host (the unshard step for a
sum-sharded value) and the final sigmoid+BCE over 256x256 values is a
negligible epilogue done in numpy.
"""

import numpy as np

N = 256
D = 64 * 64 * 64  # 262144
NCORES = 8
DLOC = D // NCORES  # 32768
P = 128  # SBUF partitions
MB = 16  # d-blocks of 128 per DMA chunk (chunk = MB*128 d-values)
NCHUNK = DLOC // (P * MB)

_built = {}
_last_results = None  # test harness reads profiling info from here


def _build(dloc=DLOC, mb=MB, compute_dtype="bf16", bufs=3):
    """Build + bacc-compile the per-core Bass kernel.

    Per-core inputs (fp32, host pre-tiled to the SBUF layout):
      f1t, f2t: [nchunk, 128, mb, N] where element [c, p, nb, i] is
      f{1,2}.reshape(N, D).T[core_off + c*128*mb + nb*128 + p, i]
    Output: out[i, j] = sum_d f1t[d, i] * f2t[d, j]   (partial Gram)
    """
    import concourse.mybir as mybir
    from concourse import bacc
    from concourse.bass import MemorySpace
    from concourse.tile import TileContext

    nchunk = dloc // (P * mb)
    assert nchunk * P * mb == dloc

    cdt = mybir.dt.bfloat16 if compute_dtype == "bf16" else mybir.dt.float32

    nc = bacc.Bacc("TRN2", target_bir_lowering=False, debug=False,
                   num_devices=NCORES)
    f1t = nc.dram_tensor("f1t", (nchunk, P, mb, N), mybir.dt.float32,
                         kind="ExternalInput")
    f2t = nc.dram_tensor("f2t", (nchunk, P, mb, N), mybir.dt.float32,
                         kind="ExternalInput")
    out = nc.dram_tensor("out", (N, N), mybir.dt.float32,
                         kind="ExternalOutput")

    f1v = f1t.ap()
    f2v = f2t.ap()

    with TileContext(nc) as tc:
        with tc.tile_pool(name="psum", bufs=1, space=MemorySpace.PSUM) as psum_pool, \
             tc.tile_pool(name="sbuf", bufs=bufs) as pool, \
             tc.tile_pool(name="outp", bufs=1) as outpool:
            acc = [psum_pool.tile([P, N], mybir.dt.float32, tag=f"acc{ib}",
                                  name=f"acc{ib}")
                   for ib in range(2)]
            for c in range(nchunk):
                # fp32 loads on the HWDGE sync ring (full HBM line rate)
                t1f = pool.tile([P, mb, N], mybir.dt.float32, tag="t1f",
                                name=f"t1f_{c}")
                t2f = pool.tile([P, mb, N], mybir.dt.float32, tag="t2f",
                                name=f"t2f_{c}")
                nc.sync.dma_start(out=t1f, in_=f1v[c])
                nc.sync.dma_start(out=t2f, in_=f2v[c])
                if cdt == mybir.dt.float32:
                    t1, t2 = t1f, t2f
                else:
                    # cast fp32->bf16 on the otherwise-idle DVE + ACT
                    t1 = pool.tile([P, mb, N], cdt, tag="t1", name=f"t1_{c}")
                    t2 = pool.tile([P, mb, N], cdt, tag="t2", name=f"t2_{c}")
                    h = mb // 2
                    nc.vector.tensor_copy(t1[:, :h], t1f[:, :h])
                    nc.scalar.copy(t1[:, h:], t1f[:, h:])
                    nc.vector.tensor_copy(t2[:, :h], t2f[:, :h])
                    nc.scalar.copy(t2[:, h:], t2f[:, h:])
                for nb in range(mb):
                    for ib in range(2):
                        nc.tensor.matmul(
                            acc[ib],
                            t1[:, nb, ib * P:(ib + 1) * P],  # lhsT [128d, 128i]
                            t2[:, nb, :],                     # rhs  [128d, 256j]
                            start=(c == 0 and nb == 0),
                            stop=(c == nchunk - 1 and nb == mb - 1),
                        )
            for ib in range(2):
                o = outpool.tile([P, N], mybir.dt.float32, tag=f"o{ib}",
                                 name=f"o{ib}")
                nc.vector.tensor_copy(o, acc[ib])
                nc.sync.dma_start(out=out.ap()[ib * P:(ib + 1) * P, :], in_=o)

    nc.compile()
    return nc


def _get_nc():
    if "nc" not in _built:
        _built["nc"] = _build()
    return _built["nc"]


def _gram_partials(in_maps, trace=False):
    global _last_results
    from concourse.bass_utils import run_bass_kernel_spmd

    nc = _get_nc()
    res = run_bass_kernel_spmd(nc, in_maps, core_ids=list(range(NCORES)),
                               trace=trace)
    _last_results = res
    return [r["out"] for r in res.results]


def _tile_layout(f, k, dloc=DLOC, mb=MB):
    """Slice core k's d-chunk of f [N, D] and pre-tile to [nchunk, P, mb, N]."""
    nchunk = dloc // (P * mb)
    x = f[:, k * dloc:(k + 1) * dloc]          # [N, dloc]
    x = x.reshape(N, nchunk, mb, P)            # d = c*(mb*P) + nb*P + p
    x = x.transpose(1, 3, 2, 0)                # [nchunk, P, mb, N]
    return np.ascontiguousarray(x)


def kernel(V1, V2):
    V1 = np.asarray(V1, dtype=np.float32)
    V2 = np.asarray(V2, dtype=np.float32)
    f1 = V1.reshape(N, D)
    f2 = V2.reshape(N, D)

    in_maps = [
        {"f1t": _tile_layout(f1, k), "f2t": _tile_layout(f2, k)}
        for k in range(NCORES)
    ]
    partials = _gram_partials(in_maps)

    Z = np.zeros((N, N), dtype=np.float64)
    for pmat in partials:
        Z += pmat
    Z /= D

    eps = 1e-12
    p = 1.0 / (1.0 + np.exp(-Z))
    p = np.clip(p, eps, 1.0 - eps)
    lab = np.eye(N, dtype=np.float64)
    loss = -np.mean(lab * np.log(p) + (1.0 - lab) * np.log1p(-p))
    return np.array(loss, dtype=np.float32)


def _selftest_sim():
    """Scaled-down correctness check in CoreSim (no hardware)."""
    from concourse.bass_interp import CoreSim

    dloc, mb = 1024, 4
    nc = _build(dloc=dloc, mb=mb)
    rng = np.random.default_rng(0)
    a = rng.standard_normal((N, dloc)).astype(np.float32)  # [N, dloc] like f1
    b = rng.standard_normal((N, dloc)).astype(np.float32)

    def tl(x):
        nchunk = dloc // (P * mb)
        return np.ascontiguousarray(
            x.reshape(N, nchunk, mb, P).transpose(1, 3, 2, 0))

    sim = CoreSim(nc)
    sim.tensor("f1t")[:] = tl(a)
    sim.tensor("f2t")[:] = tl(b)
    sim.simulate()
    got = np.array(sim.tensor("out"))
    want = a.astype(np.float64) @ b.astype(np.float64).T
    err = np.abs(got - want).max() / np.abs(want).max()
    print("selftest rel err:", err)
    assert err < 2e-2, err
    print("SELFTEST PASSED")


if __name__ == "__main__":
    _selftest_sim()


# revision 7
# speedup vs baseline: 1.0181x; 1.0181x over previous
"""CPC spatial BCE loss kernel for 8 TRN2 NeuronCores.

Computation: loss = BCE(sigmoid((V1.reshape(N,D) @ V2.reshape(N,D).T) / D), eye(N))
with N=256, D=64*64*64=262144.

Strategy (memory-regime): shard the contraction dim D across the 8 cores
(64 MB of fp32 input per core -- the minimal possible HBM traffic). Each
core computes a partial Gram matrix [256, 256] over its 32768-wide slice
of D via TensorE matmuls accumulated in fp32 PSUM. The host lays out each
core's chunk d-major and pre-tiled ([nchunk, 128, mb, 256], i.e. exactly
the SBUF tile layout) so every chunk DMA is one fully-contiguous read
with the contraction dim landing on SBUF partitions -- no on-device
transposes. Loads are SWDGE cast-DMAs (fp32 HBM -> bf16 SBUF) so the
matmuls run single-pass bf16 (fp32 matmul on trn2 costs 4x: two HI/LO
passes at half streaming rate); accumulation stays fp32 in PSUM, so the
loss error is ~1e-5, far inside tolerance.

The partial Gram matrices are summed on the host (the unshard step for a
sum-sharded value) and the final sigmoid+BCE over 256x256 values is a
negligible epilogue done in numpy.
"""

import numpy as np

N = 256
D = 64 * 64 * 64  # 262144
NCORES = 8
DLOC = D // NCORES  # 32768
P = 128  # SBUF partitions
MB = 16  # d-blocks of 128 per DMA chunk (chunk = MB*128 d-values)
NCHUNK = DLOC // (P * MB)

_built = {}
_last_results = None  # test harness reads profiling info from here


def _build(dloc=DLOC, mb=MB, compute_dtype="bf16", bufs=3):
    """Build + bacc-compile the per-core Bass kernel.

    Per-core inputs (fp32, host pre-tiled to the SBUF layout):
      f1t, f2t: [nchunk, 128, mb, N] where element [c, p, nb, i] is
      f{1,2}.reshape(N, D).T[core_off + c*128*mb + nb*128 + p, i]
    Output: out[i, j] = sum_d f1t[d, i] * f2t[d, j]   (partial Gram)
    """
    import concourse.mybir as mybir
    from concourse import bacc
    from concourse.bass import MemorySpace
    from concourse.tile import TileContext

    nchunk = dloc // (P * mb)
    assert nchunk * P * mb == dloc

    cdt = mybir.dt.bfloat16 if compute_dtype == "bf16" else mybir.dt.float32

    nc = bacc.Bacc("TRN2", target_bir_lowering=False, debug=False,
                   num_devices=NCORES)
    f1t = nc.dram_tensor("f1t", (nchunk, P, mb, N), mybir.dt.float32,
                         kind="ExternalInput")
    f2t = nc.dram_tensor("f2t", (nchunk, P, mb, N), mybir.dt.float32,
                         kind="ExternalInput")
    out = nc.dram_tensor("out", (N, N), mybir.dt.float32,
                         kind="ExternalOutput")

    f1v = f1t.ap()
    f2v = f2t.ap()

    with TileContext(nc) as tc:
        with tc.tile_pool(name="psum", bufs=1, space=MemorySpace.PSUM) as psum_pool, \
             tc.tile_pool(name="sbuf", bufs=bufs) as pool, \
             tc.tile_pool(name="outp", bufs=1) as outpool:
            acc = [psum_pool.tile([P, N], mybir.dt.float32, tag=f"acc{ib}",
                                  name=f"acc{ib}")
                   for ib in range(2)]
            for c in range(nchunk):
                # fp32 loads on the HWDGE sync ring (full HBM line rate)
                t1f = pool.tile([P, mb, N], mybir.dt.float32, tag="t1f",
                                name=f"t1f_{c}")
                t2f = pool.tile([P, mb, N], mybir.dt.float32, tag="t2f",
                                name=f"t2f_{c}")
                # two HWDGE rings (SP + ACT) stream concurrently -> HBM line rate
                nc.sync.dma_start(out=t1f, in_=f1v[c])
                nc.scalar.dma_start(out=t2f, in_=f2v[c])
                if cdt == mybir.dt.float32:
                    t1, t2 = t1f, t2f
                else:
                    # cast fp32->bf16 on the otherwise-idle DVE + ACT
                    # (DVE ~1.2ns/elem, ACT ~2.0ns/elem -> 10:6 split)
                    t1 = pool.tile([P, mb, N], cdt, tag="t1", name=f"t1_{c}")
                    t2 = pool.tile([P, mb, N], cdt, tag="t2", name=f"t2_{c}")
                    h = (mb * 5) // 8
                    nc.vector.tensor_copy(t1[:, :h], t1f[:, :h])
                    nc.scalar.copy(t1[:, h:], t1f[:, h:])
                    nc.vector.tensor_copy(t2[:, :h], t2f[:, :h])
                    nc.scalar.copy(t2[:, h:], t2f[:, h:])
                for nb in range(mb):
                    for ib in range(2):
                        nc.tensor.matmul(
                            acc[ib],
                            t1[:, nb, ib * P:(ib + 1) * P],  # lhsT [128d, 128i]
                            t2[:, nb, :],                     # rhs  [128d, 256j]
                            start=(c == 0 and nb == 0),
                            stop=(c == nchunk - 1 and nb == mb - 1),
                        )
            for ib in range(2):
                o = outpool.tile([P, N], mybir.dt.float32, tag=f"o{ib}",
                                 name=f"o{ib}")
                nc.vector.tensor_copy(o, acc[ib])
                nc.sync.dma_start(out=out.ap()[ib * P:(ib + 1) * P, :], in_=o)

    nc.compile()
    return nc


def _get_nc():
    if "nc" not in _built:
        _built["nc"] = _build()
    return _built["nc"]


def _gram_partials(in_maps, trace=False):
    global _last_results
    from concourse.bass_utils import run_bass_kernel_spmd

    nc = _get_nc()
    res = run_bass_kernel_spmd(nc, in_maps, core_ids=list(range(NCORES)),
                               trace=trace)
    _last_results = res
    return [r["out"] for r in res.results]


def _tile_layout(f, k, dloc=DLOC, mb=MB):
    """Slice core k's d-chunk of f [N, D] and pre-tile to [nchunk, P, mb, N]."""
    nchunk = dloc // (P * mb)
    x = f[:, k * dloc:(k + 1) * dloc]          # [N, dloc]
    x = x.reshape(N, nchunk, mb, P)            # d = c*(mb*P) + nb*P + p
    x = x.transpose(1, 3, 2, 0)                # [nchunk, P, mb, N]
    return np.ascontiguousarray(x)


def kernel(V1, V2):
    V1 = np.asarray(V1, dtype=np.float32)
    V2 = np.asarray(V2, dtype=np.float32)
    f1 = V1.reshape(N, D)
    f2 = V2.reshape(N, D)

    in_maps = [
        {"f1t": _tile_layout(f1, k), "f2t": _tile_layout(f2, k)}
        for k in range(NCORES)
    ]
    partials = _gram_partials(in_maps)

    Z = np.zeros((N, N), dtype=np.float64)
    for pmat in partials:
        Z += pmat
    Z /= D

    eps = 1e-12
    p = 1.0 / (1.0 + np.exp(-Z))
    p = np.clip(p, eps, 1.0 - eps)
    lab = np.eye(N, dtype=np.float64)
    loss = -np.mean(lab * np.log(p) + (1.0 - lab) * np.log1p(-p))
    return np.array(loss, dtype=np.float32)


def _selftest_sim():
    """Scaled-down correctness check in CoreSim (no hardware)."""
    from concourse.bass_interp import CoreSim

    dloc, mb = 1024, 4
    nc = _build(dloc=dloc, mb=mb)
    rng = np.random.default_rng(0)
    a = rng.standard_normal((N, dloc)).astype(np.float32)  # [N, dloc] like f1
    b = rng.standard_normal((N, dloc)).astype(np.float32)

    def tl(x):
        nchunk = dloc // (P * mb)
        return np.ascontiguousarray(
            x.reshape(N, nchunk, mb, P).transpose(1, 3, 2, 0))

    sim = CoreSim(nc)
    sim.tensor("f1t")[:] = tl(a)
    sim.tensor("f2t")[:] = tl(b)
    sim.simulate()
    got = np.array(sim.tensor("out"))
    want = a.astype(np.float64) @ b.astype(np.float64).T
    err = np.abs(got - want).max() / np.abs(want).max()
    print("selftest rel err:", err)
    assert err < 2e-2, err
    print("SELFTEST PASSED")


if __name__ == "__main__":
    _selftest_sim()
